# revision 3
# baseline (speedup 1.0000x reference)
"""Trainium2 Bass kernel for NaiveRNN.

Reference computation:
    xi = x @ W_i2h + b_i2h                      # [B, L, D_h]
    h_{t+1} = tanh(xi_t + h_t @ W_h2h + b_h2h)  # L sequential steps
    out = h_L @ W_out + b_out                   # [B, D_out]

Sharding: data-parallel over batch B=128 across 8 cores (16 rows each).
Weights replicated. No cross-core communication.

Per-core design (bf16 on the PE, f32 PSUM accumulation), one fused loop
(no separate xi phase, no xi DRAM roundtrip, no inject matmuls):

  Each step's pre-activation z_t = xi_t + h_t @ W_h2h + (b_i2h + b_h2h)
  accumulates in PSUM from three matmul families, all col-tiled 4-way
  (batch 16 uses only 16 of the PE's 128 stationary columns, so 4
  concurrent 32-col tiles stream 4 disjoint quarters of the N columns):

    X-block (independent of h, used as pipeline filler between steps):
      4 k-rounds with stationary xTT[t,k] = x_t^T chunk [128,16]
      streaming W_i2h[k, :] quarters, plus one bias round (stationary
      e0*ones16, streaming a zero-padded combined-bias row).
    h-rounds: 8 k-rounds with stationary hT chunk [128,16] streaming
      W_h2h[k, :] quarters.

  The 1024 N-columns are split into halves A (dh 0..511) and B
  (512..1023) living in SEPARATE PSUM banks (PE-write + engine-read of
  one bank is fatal), so tanh of half A runs while the PE still
  accumulates half B.  Per-step emission order
      X(t+1), hA k0-3, hB k0-3, hA k4-7, tailA, hB k4-7, tailB
  maximizes the time between a hT chunk's production (tail of step t)
  and its consumption (h-round k of step t+1).  Tails = ScalarE tanh
  (pieces of 1cc+3cc so the first-needed chunk lands earliest) + DVE
  in-place 32x32 block transpose, which with the (dh//32)%4-interleaved
  column->tile assignment yields exactly the next step's stationary
  layout (hT[p, cc, b]).

  x arrives host-pre-transposed as xTT[t, k, p, b] = x[b, t, 128k+p] so
  each step's stationary slices are one contiguous 16KB cast-DMA.
"""

import numpy as np

B, L, D_IN, D_H, D_OUT = 128, 512, 512, 1024, 512
NCORES = 8
BL = B // NCORES            # 16 local batch rows
KI = D_IN // 128            # 4 k-chunks for the x projection
KH = D_H // 128             # 8 k-chunks for the recurrence
NT = 4                      # column tiles (PE 128x32 col-tiling mode)
XPREF = 6                   # xTT prefetch depth (steps)

# tanh/transpose tail pieces, in cc (=128-col dh chunk) units:
# half A = cc 0..3, half B = cc 4..7; first piece of each half is a
# single cc so the first-needed stationary chunk is produced earliest.
PIECES = [(0, 1), (1, 4), (4, 5), (5, 8)]
PIECE_OF = {k: pi for pi, (c0, c1) in enumerate(PIECES) for k in range(c0, c1)}


def build_nc(l_steps=L):
    import concourse.bass as bass
    import concourse.mybir as mybir
    from concourse import bacc
    from concourse.tile import TileContext

    dt = mybir.dt
    f32, bf16 = dt.float32, dt.bfloat16
    AF = mybir.ActivationFunctionType

    nc = bacc.Bacc(
        "TRN2", target_bir_lowering=False, debug=False, num_devices=NCORES
    )
    # host layout: xTT[t, k, p, b] = x[b, t, 128k+p], flattened rows.
    xTT_dram = nc.dram_tensor(
        "xTT", [l_steps * D_IN, BL], f32, kind="ExternalInput"
    )
    W_i2h = nc.dram_tensor("W_i2h", [D_IN, D_H], f32, kind="ExternalInput")
    b_i2h = nc.dram_tensor("b_i2h", [D_H], f32, kind="ExternalInput")
    W_h2h = nc.dram_tensor("W_h2h", [D_H, D_H], f32, kind="ExternalInput")
    b_h2h = nc.dram_tensor("b_h2h", [D_H], f32, kind="ExternalInput")
    W_out = nc.dram_tensor("W_out", [D_H, D_OUT], f32, kind="ExternalInput")
    b_out = nc.dram_tensor("b_out", [D_OUT], f32, kind="ExternalInput")
    out = nc.dram_tensor("out", [BL, D_OUT], f32, kind="ExternalOutput")

    with TileContext(nc) as tc:
        with (
            tc.tile_pool(name="const", bufs=1) as cpool,
            tc.tile_pool(name="xr", bufs=XPREF + 2) as xpool,
            tc.tile_pool(name="hn", bufs=2) as hpool,
            tc.tile_pool(name="ps", bufs=1, space="PSUM") as pspool,
        ):
            # ---- persistent weights/constants ----
            # layouts [p, k, cc, j, w] are bit-identical to plain
            # [p, k, n]; slicing [:, k, :, j, :] gives col-tile j's
            # interleaved N-quarter stream.
            wi2h = cpool.tile([128, KI, KH, NT, 32], bf16, tag="wi2h")
            nc.gpsimd.dma_start(
                wi2h[:].rearrange("p k a j w -> p k (a j w)"),
                W_i2h.ap().rearrange("(k p) n -> p k n", p=128),
            )
            whh = cpool.tile([128, KH, KH, NT, 32], bf16, tag="whh")
            nc.gpsimd.dma_start(
                whh[:].rearrange("p k a j w -> p k (a j w)"),
                W_h2h.ap().rearrange("(k p) n -> p k n", p=128),
            )
            # W_out: f32 on the sync ring (no cast), converted once on DVE.
            wout_f = cpool.tile([128, KH, D_OUT], f32, tag="wout_f")
            nc.sync.dma_start(
                wout_f[:], W_out.ap().rearrange("(k p) n -> p k n", p=128)
            )
            wout = cpool.tile([128, KH, D_OUT], bf16, tag="wout")

            # stationary for the bias round: row 0 = ones over 16 cols.
            ones16 = cpool.tile([128, BL], bf16, tag="ones16")
            nc.gpsimd.memset(ones16[:], 0.0)
            nc.gpsimd.memset(ones16[:1, :], 1.0)
            # bias stream: row 0 = b_i2h + b_h2h (interleaved layout ==
            # plain), rows 1..127 zero (finite: junk x 0 would be NaN).
            bi = cpool.tile([1, D_H], f32, tag="bi")
            nc.sync.dma_start(bi[:], b_i2h.ap().unsqueeze(0))
            bh = cpool.tile([1, D_H], f32, tag="bh")
            nc.sync.dma_start(bh[:], b_h2h.ap().unsqueeze(0))
            bstr = cpool.tile([128, KH, NT, 32], bf16, tag="bstr")
            nc.gpsimd.memset(bstr[:], 0.0)
            nc.vector.tensor_add(
                bstr[:1].rearrange("p a j w -> p (a j w)"), bi[:], bh[:]
            )
            # phase-3 bias bits
            bo_f = cpool.tile([1, D_OUT], f32, tag="bo_f")
            nc.sync.dma_start(bo_f[:], b_out.ap().unsqueeze(0))
            bo = cpool.tile([1, D_OUT], bf16, tag="bo")
            nc.vector.tensor_copy(bo[:], bo_f[:])
            ones_row = cpool.tile([1, 128], bf16, tag="ones")
            nc.gpsimd.memset(ones_row[:], 1.0)

            # one-time wout cast (DVE idle during early steps)
            nc.vector.tensor_copy(
                wout[:].rearrange("p k n -> p (k n)"),
                wout_f[:].rearrange("p k n -> p (k n)"),
            )

            # transposed state, ping-pong x 4 pieces. hT[par][pi][p,cc,b]
            hT = [
                [
                    cpool.tile(
                        [128, c1 - c0, 32], bf16,
                        tag=f"hT{par}{pi}", name=f"hT{par}{pi}",
                    )
                    for pi, (c0, c1) in enumerate(PIECES)
                ]
                for par in range(2)
            ]
            for par in range(2):
                for pi in range(len(PIECES)):
                    nc.gpsimd.memset(hT[par][pi][:], 0.0)

            # PSUM: halves A/B x ping-pong, each a full [128,512] f32
            # bank so no two live tiles share a bank (PE-W + engine-R of
            # one bank is fatal); only cols 0..127 are used.
            zps = [
                [
                    pspool.tile([128, 512], f32, tag=f"zp{h}{par}",
                                name=f"zp{h}{par}")
                    for par in range(2)
                ]
                for h in range(2)
            ]
            # zero once: junk rows (16..31 of each 32-group) stay finite
            # forever (matmuls only ever write 16 rows per group).
            zeros_sb = cpool.tile([128, 128], f32, tag="zeros_sb")
            nc.gpsimd.memset(zeros_sb[:], 0.0)
            for zh in zps:
                for z_ in zh:
                    nc.vector.tensor_copy(z_[:, :128], zeros_sb[:])

            # xTT ring
            xts = [
                cpool.tile([128, KI, BL], bf16, tag=f"xt{i}", name=f"xt{i}")
                for i in range(XPREF)
            ]

            def emit_xtt_load(t):
                nc.gpsimd.dma_start(
                    xts[t % XPREF][:],
                    xTT_dram.ap()[D_IN * t : D_IN * (t + 1), :].rearrange(
                        "(k p) b -> p k b", p=128
                    ),
                )

            def emit_xblock(t):
                # x k-rounds + bias round for step t: fully independent
                # of the recurrence, so they act as the pipeline filler
                # between consecutive steps' h-rounds.
                xt = xts[t % XPREF]
                par = t % 2
                for h in range(2):
                    zp = zps[h][par]
                    cs = slice(4 * h, 4 * h + 4)
                    for k in range(KI):
                        for j in range(NT):
                            nc.tensor.matmul(
                                zp[32 * j : 32 * j + BL, :128],
                                xt[:, k, :],
                                wi2h[:, k, cs, j, :],
                                start=(k == 0),
                                stop=False,
                                tile_position=(0, 32 * j),
                            )
                    for j in range(NT):
                        nc.tensor.matmul(
                            zp[32 * j : 32 * j + BL, :128],
                            ones16[:],
                            bstr[:, cs, j, :],
                            start=False,
                            stop=False,
                            tile_position=(0, 32 * j),
                        )

            def emit_hrounds(t, h, ks):
                zp = zps[h][t % 2]
                hT_cur = hT[t % 2]
                cs = slice(4 * h, 4 * h + 4)
                for k in ks:
                    pi = PIECE_OF[k]
                    lhsT = hT_cur[pi][:, k - PIECES[pi][0], :BL]
                    for j in range(NT):
                        nc.tensor.matmul(
                            zp[32 * j : 32 * j + BL, :128],
                            lhsT,
                            whh[:, k, cs, j, :],
                            start=False,
                            stop=(k == KH - 1),
                            tile_position=(0, 32 * j),
                        )

            def emit_tail(t, h):
                # tanh + in-place 32x32 block transpose for half h of
                # step t, pieced so the first-needed hT chunk (cc 4h)
                # unblocks the next step's round k=4h earliest.
                zp = zps[h][t % 2]
                hT_nxt = hT[(t + 1) % 2]
                h_new = hpool.tile(
                    [128, 4, 32], bf16, tag=f"hnew{h}", name=f"hnew{h}_{t}"
                )
                for pi in (2 * h, 2 * h + 1):
                    c0, c1 = PIECES[pi]
                    lo, hi = c0 - 4 * h, c1 - 4 * h
                    nc.scalar.activation(
                        h_new[:, lo:hi, :].rearrange("p a w -> p (a w)"),
                        zp[:, 32 * lo : 32 * hi],
                        AF.Tanh,
                    )
                    nc.vector.transpose(
                        hT_nxt[pi][:].rearrange("p a w -> p (a w)"),
                        h_new[:, lo:hi, :].rearrange("p a w -> p (a w)"),
                    )

            # ---- pipeline ----
            for t in range(min(XPREF, l_steps)):
                emit_xtt_load(t)
            emit_xblock(0)
            for t in range(l_steps):
                if t + XPREF < l_steps:
                    emit_xtt_load(t + XPREF)
                if t + 1 < l_steps:
                    emit_xblock(t + 1)
                emit_hrounds(t, 0, range(4))
                emit_hrounds(t, 1, range(4))
                emit_hrounds(t, 0, range(4, KH))
                emit_tail(t, 0)
                emit_hrounds(t, 1, range(4, KH))
                emit_tail(t, 1)

            # ---- head: out = h_L @ W_out + b_out ----
            zp3 = pspool.tile([128, D_OUT], f32, tag="zp3")
            hT_fin = hT[l_steps % 2]
            nc.tensor.matmul(
                zp3[:BL, :], ones_row[:, :BL], bo[:],
                start=True, stop=False, tile_position=(0, 0),
            )
            for k in range(KH):
                pi = PIECE_OF[k]
                nc.tensor.matmul(
                    zp3[:BL, :],
                    hT_fin[pi][:, k - PIECES[pi][0], :BL],
                    wout[:, k, :],
                    start=False,
                    stop=(k == KH - 1),
                    tile_position=(0, 0),
                )
            out_sb = cpool.tile([128, D_OUT], f32, tag="out_sb")
            nc.vector.tensor_copy(out_sb[:BL], zp3[:BL])
            nc.sync.dma_start(out.ap()[:], out_sb[:BL])

    nc.compile()
    return nc


_CACHE = {}


def _get_nc(l_steps=L):
    if l_steps not in _CACHE:
        _CACHE[l_steps] = build_nc(l_steps)
    return _CACHE[l_steps]


def run(inputs, l_steps=L, trace=False, tmpdir=None):
    from concourse.bass_utils import run_bass_kernel_spmd

    nc = _get_nc(l_steps)
    x = np.asarray(inputs["x"], np.float32).reshape(B, L, D_IN)
    shared = {
        k: np.ascontiguousarray(np.asarray(inputs[k], np.float32))
        for k in ("W_i2h", "b_i2h", "W_h2h", "b_h2h", "W_out", "b_out")
    }
    in_maps = []
    for c in range(NCORES):
        m = dict(shared)
        xl = x[c * BL : (c + 1) * BL, :l_steps]  # [16, l, 512]
        # xTT[t, k, p, b] = x[b, t, 128k+p]
        m["xTT"] = np.ascontiguousarray(xl.transpose(1, 2, 0)).reshape(
            l_steps * D_IN, BL
        )
        in_maps.append(m)
    res = run_bass_kernel_spmd(
        nc,
        in_maps,
        core_ids=list(range(NCORES)),
        trace=trace,
        tmpdir=tmpdir,
    )
    out = np.concatenate([r["out"] for r in res.results], axis=0)
    return out, res


def kernel(**inputs) -> np.ndarray:
    out, _ = run(inputs)
    return out


# revision 6
# speedup vs baseline: 1.2586x; 1.2586x over previous
"""Trainium2 Bass kernel for NaiveRNN (raw bass, manual semaphores).

Reference computation:
    xi = x @ W_i2h + b_i2h                      # [B, L, D_h]
    h_{t+1} = tanh(xi_t + h_t @ W_h2h + b_h2h)  # L sequential steps
    out = h_L @ W_out + b_out                   # [B, D_out]

Sharding: data-parallel over batch B=128 across 8 cores (16 rows each).
Weights replicated; no cross-core communication.

Why raw bass: under the Tile framework every matmul carries a
serialized ~26-30ns semaphore increment (EVT_SEM register write), which
caps the PE at ~34ns/matmul; with 104 matmuls per recurrence step that
is 3.5us/step against a 1.38us streaming floor.  Here each engine runs
a hand-scheduled program with 2 increments per step on the PE and
explicit waits, so matmul issue returns to the ~4-6ns NX rate and the
step time approaches the W_h2h + W_i2h streaming floor.

Structure per step t (one batch-16 group, 4-way PE column tiling, all
operands bf16, f32 PSUM):
  z_t = xi_t + h_t @ W_h2h + (b_i2h+b_h2h) accumulates in two PSUM
  half-banks A (dh cols 0..511) and B (512..1023), rotating over 3 bank
  pairs (par = t%3) so nothing ever waits on a previous step's tanh.
    X-block(t): 4 x-rounds (stationary xTT[t,k] = x_t^T chunk [128,16])
      + 1 bias round, each 8 matmuls (4 col-tiles x 2 half-banks, N=128)
    h-rounds: 8 k-rounds (stationary hT chunk [128,16]) x 2 halves.
  PE order: X(t+1) | hA k0-3 | hB k0-3 | (wait chunks 4-7) hA k4-7 |
  hB k4-7, with s_pe incremented after hA k7 and hB k7.  ScalarE tanh
  of half A runs while the PE accumulates half B / next X-block; DVE
  does the in-place 32x32 block transpose (interleaved (dh//32)%4
  column->tile assignment makes that exactly the next stationary
  layout).  Chain: PE -> s_pe -> ACT -> s_act -> DVE -> s_dve -> PE.

  x arrives host-pre-transposed as xTT[t, k, p, b] = x[b, t, 128k+p];
  per-step 32KB cast-DMAs on the gpsimd (SWDGE) ring, 6 deep.
"""

import numpy as np

B, L, D_IN, D_H, D_OUT = 128, 512, 512, 1024, 512
NCORES = 8
BL = B // NCORES            # 16 local batch rows
KI = D_IN // 128            # 4 k-chunks for the x projection
KH = D_H // 128             # 8 k-chunks for the recurrence
NT = 4                      # column tiles (PE 128x32 col-tiling mode)
XPREF = 6                   # xTT prefetch depth (steps)
NPAR = 3                    # psum bank-pair rotation depth


def build_nc(l_steps=L):
    import concourse.bass as bass
    import concourse.mybir as mybir
    from concourse import bacc
    from contextlib import ExitStack

    dt = mybir.dt
    f32, bf16 = dt.float32, dt.bfloat16
    AF = mybir.ActivationFunctionType

    nc = bacc.Bacc(
        "TRN2", target_bir_lowering=False, debug=False, num_devices=NCORES
    )
    xTT_dram = nc.dram_tensor(
        "xTT", [l_steps * D_IN, BL], f32, kind="ExternalInput"
    )
    W_i2h = nc.dram_tensor("W_i2h", [D_IN, D_H], f32, kind="ExternalInput")
    b_i2h = nc.dram_tensor("b_i2h", [D_H], f32, kind="ExternalInput")
    W_h2h = nc.dram_tensor("W_h2h", [D_H, D_H], f32, kind="ExternalInput")
    b_h2h = nc.dram_tensor("b_h2h", [D_H], f32, kind="ExternalInput")
    W_out = nc.dram_tensor("W_out", [D_H, D_OUT], f32, kind="ExternalInput")
    b_out = nc.dram_tensor("b_out", [D_OUT], f32, kind="ExternalInput")
    out = nc.dram_tensor("out", [BL, D_OUT], f32, kind="ExternalOutput")

    es = ExitStack()
    with es:
        # ---- SBUF tensors ----
        # [p, k, cc, j, w] layouts are bit-identical to plain [p, k, n];
        # slicing [:, k, ccs, j, :] gives col-tile j's interleaved
        # N-columns stream for an (accumulation-chunk, half) round.
        sb = lambda name, shape, dtype: es.enter_context(
            nc.sbuf_tensor(name, shape, dtype)
        )
        wi2h = sb("wi2h", [128, KI, KH, NT, 32], bf16)
        whh = sb("whh", [128, KH, KH, NT, 32], bf16)
        wout_f = sb("wout_f", [128, KH, D_OUT], f32)
        wout = sb("wout", [128, KH, D_OUT], bf16)
        ones16 = sb("ones16", [128, BL], bf16)
        bi = sb("bi", [1, D_H], f32)
        bh = sb("bh", [1, D_H], f32)
        bstr = sb("bstr", [128, KH, NT, 32], bf16)
        bo_f = sb("bo_f", [1, D_OUT], f32)
        bo = sb("bo", [1, D_OUT], bf16)
        ones_row = sb("ones_row", [1, 128], bf16)
        out_sb = sb("out_sb", [128, D_OUT], f32)
        # transposed state, ping-pong: hT[par][p, cc, b-slot]
        hT = [sb(f"hT{i}", [128, KH, 32], bf16) for i in range(2)]
        # tanh outputs, ping-pong per half
        hnA = [sb(f"hnA{i}", [128, 128], bf16) for i in range(2)]
        hnB = [sb(f"hnB{i}", [128, 128], bf16) for i in range(2)]
        xts = [sb(f"xt{i}", [128, KI, BL], bf16) for i in range(XPREF)]
        # ---- PSUM: one full bank per tensor (no bank sharing) ----
        zpA = [
            es.enter_context(nc.psum_tensor(f"zpA{i}", [128, 512], f32))
            for i in range(NPAR)
        ]
        zpB = [
            es.enter_context(nc.psum_tensor(f"zpB{i}", [128, 512], f32))
            for i in range(NPAR)
        ]
        zp3 = es.enter_context(nc.psum_tensor("zp3", [128, D_OUT], f32))

        s_dg = es.enter_context(nc.semaphore("s_dg"))   # gpsimd DMA ring
        s_ds = es.enter_context(nc.semaphore("s_ds"))   # sync DMA ring
        s_gs = es.enter_context(nc.semaphore("s_gs"))   # gpsimd setup done
        s_vs = es.enter_context(nc.semaphore("s_vs"))   # vector setup
        s_pe = es.enter_context(nc.semaphore("s_pe"))   # PE half-pass done
        s_act = es.enter_context(nc.semaphore("s_act"))  # tanh done
        s_dve = es.enter_context(nc.semaphore("s_dve"))  # transpose done
        all_sems = [s_dg, s_ds, s_gs, s_vs, s_pe, s_act, s_dve]

        # semaphores are not cleared on allocation: reset them, then
        # barrier so no engine can race past a stale value.
        for s_ in all_sems:
            nc.gpsimd.dma_reset(range(s_.num, s_.num + 1))
            nc.gpsimd.sem_clear(s_)
        nc.all_engine_barrier()

        ndg = [0]  # gpsimd DMA count (emission-time bookkeeping)
        dg_x = {}  # step -> s_dg threshold when its xTT tile is loaded

        with nc.Block() as block:

            @block.gpsimd
            def _(g):
                # constants
                g.memset(ones16[:], 0.0)
                g.memset(ones16[:1, :], 1.0)
                g.memset(bstr[:], 0.0)
                g.memset(ones_row[:], 1.0)
                g.memset(hT[0][:], 0.0)
                g.memset(hT[1][:], 0.0).then_inc(s_gs, 1)
                # weights (SWDGE casts f32 -> bf16)
                g.dma_start(
                    wi2h[:].rearrange("p k a j w -> p k (a j w)"),
                    W_i2h.ap().rearrange("(k p) n -> p k n", p=128),
                ).then_inc(s_dg, 16)
                ndg[0] += 1
                g.dma_start(
                    whh[:].rearrange("p k a j w -> p k (a j w)"),
                    W_h2h.ap().rearrange("(k p) n -> p k n", p=128),
                ).then_inc(s_dg, 16)
                ndg[0] += 1

                def load_x(t):
                    g.dma_start(
                        xts[t % XPREF][:],
                        xTT_dram.ap()[D_IN * t : D_IN * (t + 1), :].rearrange(
                            "(k p) b -> p k b", p=128
                        ),
                    ).then_inc(s_dg, 16)
                    ndg[0] += 1
                    dg_x[t] = 16 * ndg[0]

                for t in range(min(XPREF, l_steps)):
                    load_x(t)
                for t in range(l_steps - XPREF):
                    # ring slot (t+XPREF)%XPREF was last read by X-block(t),
                    # which completes before the s_pe inc 2t+1.
                    g.wait_ge(s_pe, 2 * t + 1)
                    load_x(t + XPREF)

            @block.sync
            def _(s):
                s.dma_start(bi[:], b_i2h.ap().unsqueeze(0)).then_inc(s_ds, 16)
                s.dma_start(bh[:], b_h2h.ap().unsqueeze(0)).then_inc(s_ds, 16)
                s.dma_start(bo_f[:], b_out.ap().unsqueeze(0)).then_inc(
                    s_ds, 16
                )
                s.dma_start(
                    wout_f[:], W_out.ap().rearrange("(k p) n -> p k n", p=128)
                ).then_inc(s_ds, 16)
                # final output; DVE signals 2*l_steps+1 after the head copy
                s.wait_ge(s_dve, 2 * l_steps + 1)
                s.dma_start(out.ap()[:], out_sb[:BL]).then_inc(s_ds, 16)

            @block.vector
            def _(v):
                # one-time: combined bias row, bias-out cast, wout cast
                v.wait_ge(s_ds, 32)
                v.wait_ge(s_gs, 1)
                nc.vector.tensor_add(
                    bstr[:1].rearrange("p a j w -> p (a j w)"), bi[:], bh[:]
                ).then_inc(s_vs, 1)
                v.wait_ge(s_ds, 48)
                nc.vector.tensor_copy(bo[:], bo_f[:])
                v.wait_ge(s_ds, 64)
                nc.vector.tensor_copy(
                    wout[:].rearrange("p k n -> p (k n)"),
                    wout_f[:].rearrange("p k n -> p (k n)"),
                ).then_inc(s_vs, 1)
                # steady state: block transposes
                for t in range(l_steps):
                    hT_nxt = hT[(t + 1) % 2]
                    v.wait_ge(s_act, 2 * t + 1)
                    nc.vector.transpose(
                        hT_nxt[:, 0:4, :].rearrange("p a w -> p (a w)"),
                        hnA[t % 2][:],
                    ).then_inc(s_dve, 1)
                    v.wait_ge(s_act, 2 * t + 2)
                    nc.vector.transpose(
                        hT_nxt[:, 4:8, :].rearrange("p a w -> p (a w)"),
                        hnB[t % 2][:],
                    ).then_inc(s_dve, 1)
                # head: copy psum to sbuf for the out-DMA
                v.wait_ge(s_pe, 2 * l_steps + 1)
                nc.vector.tensor_copy(out_sb[:BL], zp3[:BL]).then_inc(
                    s_dve, 1
                )

            @block.scalar
            def _(sc):
                for t in range(l_steps):
                    sc.wait_ge(s_pe, 2 * t + 1)
                    nc.scalar.activation(
                        hnA[t % 2][:], zpA[t % NPAR][:, :128], AF.Tanh
                    ).then_inc(s_act, 1)
                    sc.wait_ge(s_pe, 2 * t + 2)
                    nc.scalar.activation(
                        hnB[t % 2][:], zpB[t % NPAR][:, :128], AF.Tanh
                    ).then_inc(s_act, 1)

            @block.tensor
            def _(pe):
                def xblock(t):
                    # x k-rounds + bias round for step t; independent of
                    # the recurrence state -> pipeline filler.
                    xt = xts[t % XPREF]
                    par = t % NPAR
                    for k in range(KI + 1):
                        for zp, cs in (
                            (zpA[par], slice(0, 4)),
                            (zpB[par], slice(4, 8)),
                        ):
                            for j in range(NT):
                                if k < KI:
                                    nc.tensor.matmul(
                                        zp[32 * j : 32 * j + BL, :128],
                                        xt[:, k, :],
                                        wi2h[:, k, cs, j, :],
                                        start=(k == 0),
                                        stop=False,
                                        tile_position=(0, 32 * j),
                                    )
                                else:
                                    nc.tensor.matmul(
                                        zp[32 * j : 32 * j + BL, :128],
                                        ones16[:],
                                        bstr[:, cs, j, :],
                                        start=False,
                                        stop=False,
                                        tile_position=(0, 32 * j),
                                    )

                def hrounds(t, half, ks):
                    # h-rounds for one psum half-bank; the last (k=7)
                    # round of each half raises s_pe for its tanh.
                    par = t % NPAR
                    hT_cur = hT[t % 2]
                    zp = (zpA, zpB)[half][par]
                    cs = slice(4 * half, 4 * half + 4)
                    for k in ks:
                        inst = None
                        for j in range(NT):
                            inst = nc.tensor.matmul(
                                zp[32 * j : 32 * j + BL, :128],
                                hT_cur[:, k, :BL],
                                whh[:, k, cs, j, :],
                                start=False,
                                stop=(k == KH - 1),
                                tile_position=(0, 32 * j),
                            )
                        if k == KH - 1:
                            inst.then_inc(s_pe, 1)

                pe.wait_ge(s_gs, 1)      # hT zeros, constants
                pe.wait_ge(s_vs, 1)      # bstr bias row
                pe.wait_ge(s_dg, dg_x[0])  # wi2h, whh, xTT[0]
                xblock(0)
                for t in range(l_steps):
                    if t + 1 < l_steps:
                        pe.wait_ge(s_dg, dg_x[t + 1])
                        xblock(t + 1)
                    if t >= 1:
                        pe.wait_ge(s_dve, 2 * t - 1)  # chunks 0-3 of h_t
                    hrounds(t, 0, range(0, 4))
                    hrounds(t, 1, range(0, 4))
                    if t >= 1:
                        pe.wait_ge(s_dve, 2 * t)      # chunks 4-7 of h_t
                    hrounds(t, 0, range(4, KH))       # A-cols final -> tanh A
                    hrounds(t, 1, range(4, KH))       # B-cols final -> tanh B

                # ---- head: out = h_L @ W_out + b_out ----
                pe.wait_ge(s_dve, 2 * l_steps)
                pe.wait_ge(s_vs, 2)      # wout cast done
                hT_fin = hT[l_steps % 2]
                nc.tensor.matmul(
                    zp3[:BL, :], ones_row[:, :BL], bo[:],
                    start=True, stop=False, tile_position=(0, 0),
                )
                for k in range(KH):
                    inst = nc.tensor.matmul(
                        zp3[:BL, :],
                        hT_fin[:, k, :BL],
                        wout[:, k, :],
                        start=False,
                        stop=(k == KH - 1),
                        tile_position=(0, 0),
                    )
                inst.then_inc(s_pe, 1)

        nc.compile()
    return nc


_CACHE = {}


def _get_nc(l_steps=L):
    if l_steps not in _CACHE:
        _CACHE[l_steps] = build_nc(l_steps)
    return _CACHE[l_steps]


def run(inputs, l_steps=L, trace=False, tmpdir=None):
    from concourse.bass_utils import run_bass_kernel_spmd

    nc = _get_nc(l_steps)
    x = np.asarray(inputs["x"], np.float32).reshape(B, L, D_IN)
    shared = {
        k: np.ascontiguousarray(np.asarray(inputs[k], np.float32))
        for k in ("W_i2h", "b_i2h", "W_h2h", "b_h2h", "W_out", "b_out")
    }
    in_maps = []
    for c in range(NCORES):
        m = dict(shared)
        xl = x[c * BL : (c + 1) * BL, :l_steps]  # [16, l, 512]
        # xTT[t, k, p, b] = x[b, t, 128k+p]
        m["xTT"] = np.ascontiguousarray(xl.transpose(1, 2, 0)).reshape(
            l_steps * D_IN, BL
        )
        in_maps.append(m)
    res = run_bass_kernel_spmd(
        nc,
        in_maps,
        core_ids=list(range(NCORES)),
        trace=trace,
        tmpdir=tmpdir,
    )
    out = np.concatenate([r["out"] for r in res.results], axis=0)
    return out, res


def kernel(**inputs) -> np.ndarray:
    out, _ = run(inputs)
    return out


# revision 13
# speedup vs baseline: 1.2921x; 1.0266x over previous
"""Trainium2 Bass kernel for NaiveRNN (raw bass, manual semaphores).

Reference computation:
    xi = x @ W_i2h + b_i2h                      # [B, L, D_h]
    h_{t+1} = tanh(xi_t + h_t @ W_h2h + b_h2h)  # L sequential steps
    out = h_L @ W_out + b_out                   # [B, D_out]

Sharding: data-parallel over batch B=128 across 8 cores (16 rows each).
Weights replicated; no cross-core communication.

Why raw bass: under the Tile framework every matmul carries a
serialized ~26-30ns semaphore increment (EVT_SEM register write), which
caps the PE at ~34ns/matmul; with 104 matmuls per recurrence step that
is 3.5us/step against a 1.38us streaming floor.  Here each engine runs
a hand-scheduled program with 2 increments per step on the PE and
explicit waits, so matmul issue returns to the ~4-6ns NX rate and the
step time approaches the W_h2h + W_i2h streaming floor.

Structure per step t (one batch-16 group, 4-way PE column tiling, all
operands bf16, f32 PSUM):
  z_t = xi_t + h_t @ W_h2h + (b_i2h+b_h2h) accumulates in two PSUM
  half-banks A (dh cols 0..511) and B (512..1023), rotating over 3 bank
  pairs (par = t%3) so nothing ever waits on a previous step's tanh.
    X-block(t): 4 x-rounds (stationary xTT[t,k] = x_t^T chunk [128,16])
      + 1 bias round, each 8 matmuls (4 col-tiles x 2 half-banks, N=128)
    h-rounds: 8 k-rounds (stationary hT chunk [128,16]) x 2 halves.
  PE order: X(t+1) | hA k0-3 | hB k0-3 | (wait chunks 4-7) hA k4-7 |
  hB k4-7, with s_pe incremented after hA k7 and hB k7.  ScalarE tanh
  of half A runs while the PE accumulates half B / next X-block; DVE
  does the in-place 32x32 block transpose (interleaved (dh//32)%4
  column->tile assignment makes that exactly the next stationary
  layout).  Chain: PE -> s_pe -> ACT -> s_act -> DVE -> s_dve -> PE.

  x arrives host-pre-transposed as xTT[t, k, p, b] = x[b, t, 128k+p];
  per-step 32KB cast-DMAs on the gpsimd (SWDGE) ring, 6 deep.
"""

import numpy as np

B, L, D_IN, D_H, D_OUT = 128, 512, 512, 1024, 512
NCORES = 8
BL = B // NCORES            # 16 local batch rows
KI = D_IN // 128            # 4 k-chunks for the x projection
KH = D_H // 128             # 8 k-chunks for the recurrence
NT = 4                      # column tiles (PE 128x32 col-tiling mode)
CHSZ = 64                   # steps per xTT chunk-DMA
NPAR = 3                    # psum bank-pair rotation depth


def build_nc(l_steps=L):
    import concourse.bass as bass
    import concourse.mybir as mybir
    from concourse import bacc
    from contextlib import ExitStack

    dt = mybir.dt
    f32, bf16 = dt.float32, dt.bfloat16
    AF = mybir.ActivationFunctionType

    nc = bacc.Bacc(
        "TRN2", target_bir_lowering=False, debug=False, num_devices=NCORES
    )
    nch = -(-l_steps // CHSZ)  # xTT chunks (host zero-pads the last one)
    # host layout XH[ch, p, tt, k, b] = x[b, CHSZ*ch+tt, 128k+p]: each
    # partition's per-chunk slice is 16KB contiguous, so every chunk DMA
    # is 128 fat descriptors (vs per-step loads, whose 32-byte packet
    # storms stalled the whole chip ~2us per step).
    xTT_dram = nc.dram_tensor(
        "xTT", [nch * 128, CHSZ * KI * BL], f32, kind="ExternalInput"
    )
    W_i2h = nc.dram_tensor("W_i2h", [D_IN, D_H], f32, kind="ExternalInput")
    b_i2h = nc.dram_tensor("b_i2h", [D_H], f32, kind="ExternalInput")
    W_h2h = nc.dram_tensor("W_h2h", [D_H, D_H], f32, kind="ExternalInput")
    b_h2h = nc.dram_tensor("b_h2h", [D_H], f32, kind="ExternalInput")
    W_out = nc.dram_tensor("W_out", [D_H, D_OUT], f32, kind="ExternalInput")
    b_out = nc.dram_tensor("b_out", [D_OUT], f32, kind="ExternalInput")
    out = nc.dram_tensor("out", [BL, D_OUT], f32, kind="ExternalOutput")

    es = ExitStack()
    with es:
        # ---- SBUF tensors ----
        # [p, k, cc, j, w] layouts are bit-identical to plain [p, k, n];
        # slicing [:, k, ccs, j, :] gives col-tile j's interleaved
        # N-columns stream for an (accumulation-chunk, half) round.
        sb = lambda name, shape, dtype: es.enter_context(
            nc.sbuf_tensor(name, shape, dtype)
        )
        wi2h = sb("wi2h", [128, KI, KH, NT, 32], bf16)
        whh = sb("whh", [128, KH, KH, NT, 32], bf16)
        wout_f = sb("wout_f", [128, KH, D_OUT], f32)
        wout = sb("wout", [128, KH, D_OUT], bf16)
        ones16 = sb("ones16", [128, BL], bf16)
        bi = sb("bi", [1, D_H], f32)
        bh = sb("bh", [1, D_H], f32)
        bstr = sb("bstr", [128, KH, NT, 32], bf16)
        bo_f = sb("bo_f", [1, D_OUT], f32)
        bo = sb("bo", [1, D_OUT], bf16)
        ones_row = sb("ones_row", [1, 128], bf16)
        out_sb = sb("out_sb", [128, D_OUT], f32)
        # transposed state, ping-pong: hT[par][p, cc, b-slot]
        hT = [sb(f"hT{i}", [128, KH, 32], bf16) for i in range(2)]
        # tanh outputs, ping-pong per half
        hnA = [sb(f"hnA{i}", [128, 128], bf16) for i in range(2)]
        hnB = [sb(f"hnB{i}", [128, 128], bf16) for i in range(2)]
        # whole x projection input stays SBUF-resident (8KB/partition
        # per 64-step chunk; 64KB/partition for the full 512 steps)
        xsb = sb("xsb", [128, nch, CHSZ, KI, BL], bf16)
        # ---- PSUM: one full bank per tensor (no bank sharing) ----
        zpA = [
            es.enter_context(nc.psum_tensor(f"zpA{i}", [128, 512], f32))
            for i in range(NPAR)
        ]
        zpB = [
            es.enter_context(nc.psum_tensor(f"zpB{i}", [128, 512], f32))
            for i in range(NPAR)
        ]
        zp3 = es.enter_context(nc.psum_tensor("zp3", [128, D_OUT], f32))

        s_dg = es.enter_context(nc.semaphore("s_dg"))   # gpsimd DMA ring
        s_ds = es.enter_context(nc.semaphore("s_ds"))   # sync DMA ring
        s_gs = es.enter_context(nc.semaphore("s_gs"))   # gpsimd setup done
        s_vs = es.enter_context(nc.semaphore("s_vs"))   # vector setup
        s_pe = es.enter_context(nc.semaphore("s_pe"))   # PE half-pass done
        s_act = es.enter_context(nc.semaphore("s_act"))  # tanh done
        s_dve = es.enter_context(nc.semaphore("s_dve"))  # transpose done
        all_sems = [s_dg, s_ds, s_gs, s_vs, s_pe, s_act, s_dve]

        # semaphores are not cleared on allocation: reset them, then
        # barrier so no engine can race past a stale value.
        for s_ in all_sems:
            nc.gpsimd.dma_reset(range(s_.num, s_.num + 1))
            nc.gpsimd.sem_clear(s_)
        nc.all_engine_barrier()

        ndg = [0]  # gpsimd DMA count (emission-time bookkeeping)
        dg_x = {}  # step -> s_dg threshold when its xTT tile is loaded

        with nc.Block() as block:

            @block.gpsimd
            def _(g):
                # constants
                g.memset(ones16[:], 0.0)
                g.memset(ones16[:1, :], 1.0)
                g.memset(bstr[:], 0.0)
                g.memset(ones_row[:], 1.0)
                g.memset(hT[0][:], 0.0)
                g.memset(hT[1][:], 0.0).then_inc(s_gs, 1)
                # weights (SWDGE casts f32 -> bf16)
                g.dma_start(
                    wi2h[:].rearrange("p k a j w -> p k (a j w)"),
                    W_i2h.ap().rearrange("(k p) n -> p k n", p=128),
                ).then_inc(s_dg, 16)
                ndg[0] += 1
                g.dma_start(
                    whh[:].rearrange("p k a j w -> p k (a j w)"),
                    W_h2h.ap().rearrange("(k p) n -> p k n", p=128),
                ).then_inc(s_dg, 16)
                ndg[0] += 1

                for ch in range(nch):
                    g.dma_start(
                        xsb[:, ch, :, :, :].rearrange(
                            "p t k b -> p (t k b)"
                        ),
                        xTT_dram.ap()[128 * ch : 128 * (ch + 1), :],
                    ).then_inc(s_dg, 16)
                    ndg[0] += 1
                    dg_x[ch] = 16 * ndg[0]

            @block.sync
            def _(s):
                s.dma_start(bi[:], b_i2h.ap().unsqueeze(0)).then_inc(s_ds, 16)
                s.dma_start(bh[:], b_h2h.ap().unsqueeze(0)).then_inc(s_ds, 16)
                s.dma_start(bo_f[:], b_out.ap().unsqueeze(0)).then_inc(
                    s_ds, 16
                )
                s.dma_start(
                    wout_f[:], W_out.ap().rearrange("(k p) n -> p k n", p=128)
                ).then_inc(s_ds, 16)
                # final output; DVE signals 2*l_steps+1 after the head copy
                s.wait_ge(s_dve, 2 * l_steps + 1)
                s.dma_start(out.ap()[:], out_sb[:BL]).then_inc(s_ds, 16)

            @block.vector
            def _(v):
                # one-time: combined bias row, bias-out cast, wout cast
                v.wait_ge(s_ds, 32)
                v.wait_ge(s_gs, 1)
                nc.vector.tensor_add(
                    bstr[:1].rearrange("p a j w -> p (a j w)"), bi[:], bh[:]
                ).then_inc(s_vs, 1)
                v.wait_ge(s_ds, 48)
                nc.vector.tensor_copy(bo[:], bo_f[:])
                v.wait_ge(s_ds, 64)
                nc.vector.tensor_copy(
                    wout[:].rearrange("p k n -> p (k n)"),
                    wout_f[:].rearrange("p k n -> p (k n)"),
                ).then_inc(s_vs, 1)
                # steady state: block transposes
                for t in range(l_steps):
                    hT_nxt = hT[(t + 1) % 2]
                    v.wait_ge(s_act, 2 * t + 1)
                    nc.vector.transpose(
                        hT_nxt[:, 0:4, :].rearrange("p a w -> p (a w)"),
                        hnA[t % 2][:],
                    ).then_inc(s_dve, 1)
                    v.wait_ge(s_act, 2 * t + 2)
                    nc.vector.transpose(
                        hT_nxt[:, 4:8, :].rearrange("p a w -> p (a w)"),
                        hnB[t % 2][:],
                    ).then_inc(s_dve, 1)
                # head: copy psum to sbuf for the out-DMA
                v.wait_ge(s_pe, 2 * l_steps + 1)
                nc.vector.tensor_copy(out_sb[:BL], zp3[:BL]).then_inc(
                    s_dve, 1
                )

            @block.scalar
            def _(sc):
                for t in range(l_steps):
                    sc.wait_ge(s_pe, 2 * t + 1)
                    nc.scalar.activation(
                        hnA[t % 2][:], zpA[t % NPAR][:, :128], AF.Tanh
                    ).then_inc(s_act, 1)
                    sc.wait_ge(s_pe, 2 * t + 2)
                    nc.scalar.activation(
                        hnB[t % 2][:], zpB[t % NPAR][:, :128], AF.Tanh
                    ).then_inc(s_act, 1)

            @block.tensor
            def _(pe):
                def xblock(t):
                    # x k-rounds + bias round for step t; independent of
                    # the recurrence state -> pipeline filler.
                    xt = xsb[:, t // CHSZ, t % CHSZ, :, :]
                    par = t % NPAR
                    for k in range(KI + 1):
                        for zp, cs in (
                            (zpA[par], slice(0, 4)),
                            (zpB[par], slice(4, 8)),
                        ):
                            for j in range(NT):
                                if k < KI:
                                    nc.tensor.matmul(
                                        zp[32 * j : 32 * j + BL, :128],
                                        xt[:, k, :],
                                        wi2h[:, k, cs, j, :],
                                        start=(k == 0),
                                        stop=False,
                                        tile_position=(0, 32 * j),
                                    )
                                else:
                                    nc.tensor.matmul(
                                        zp[32 * j : 32 * j + BL, :128],
                                        ones16[:],
                                        bstr[:, cs, j, :],
                                        start=False,
                                        stop=False,
                                        tile_position=(0, 32 * j),
                                    )

                def hrounds(t, half, ks):
                    # h-rounds for one psum half-bank; the last (k=7)
                    # round of each half raises s_pe for its tanh.
                    par = t % NPAR
                    hT_cur = hT[t % 2]
                    zp = (zpA, zpB)[half][par]
                    cs = slice(4 * half, 4 * half + 4)
                    for k in ks:
                        inst = None
                        for j in range(NT):
                            inst = nc.tensor.matmul(
                                zp[32 * j : 32 * j + BL, :128],
                                hT_cur[:, k, :BL],
                                whh[:, k, cs, j, :],
                                start=False,
                                stop=(k == KH - 1),
                                tile_position=(0, 32 * j),
                            )
                        if k == KH - 1:
                            inst.then_inc(s_pe, 1)

                pe.wait_ge(s_gs, 1)      # hT zeros, constants
                pe.wait_ge(s_vs, 1)      # bstr bias row
                pe.wait_ge(s_dg, dg_x[0])  # wi2h, whh, xTT chunk 0
                xblock(0)
                for t in range(l_steps):
                    if t + 1 < l_steps:
                        if (t + 1) % CHSZ == 0:
                            pe.wait_ge(s_dg, dg_x[(t + 1) // CHSZ])
                        xblock(t + 1)
                    if t >= 1:
                        pe.wait_ge(s_dve, 2 * t - 1)  # chunks 0-3 of h_t
                    hrounds(t, 0, range(0, 4))
                    hrounds(t, 1, range(0, 4))
                    if t >= 1:
                        pe.wait_ge(s_dve, 2 * t)      # chunks 4-7 of h_t
                    hrounds(t, 0, range(4, KH))       # A-cols final -> tanh A
                    hrounds(t, 1, range(4, KH))       # B-cols final -> tanh B

                # ---- head: out = h_L @ W_out + b_out ----
                pe.wait_ge(s_dve, 2 * l_steps)
                pe.wait_ge(s_vs, 2)      # wout cast done
                hT_fin = hT[l_steps % 2]
                nc.tensor.matmul(
                    zp3[:BL, :], ones_row[:, :BL], bo[:],
                    start=True, stop=False, tile_position=(0, 0),
                )
                for k in range(KH):
                    inst = nc.tensor.matmul(
                        zp3[:BL, :],
                        hT_fin[:, k, :BL],
                        wout[:, k, :],
                        start=False,
                        stop=(k == KH - 1),
                        tile_position=(0, 0),
                    )
                inst.then_inc(s_pe, 1)

        nc.compile()
    return nc


_CACHE = {}


def _get_nc(l_steps=L):
    if l_steps not in _CACHE:
        _CACHE[l_steps] = build_nc(l_steps)
    return _CACHE[l_steps]


def run(inputs, l_steps=L, trace=False, tmpdir=None):
    from concourse.bass_utils import run_bass_kernel_spmd

    nc = _get_nc(l_steps)
    x = np.asarray(inputs["x"], np.float32).reshape(B, L, D_IN)
    shared = {
        k: np.ascontiguousarray(np.asarray(inputs[k], np.float32))
        for k in ("W_i2h", "b_i2h", "W_h2h", "b_h2h", "W_out", "b_out")
    }
    nch = -(-l_steps // CHSZ)
    in_maps = []
    for c in range(NCORES):
        m = dict(shared)
        xl = x[c * BL : (c + 1) * BL, :l_steps]  # [16, l, 512]
        # XH[ch, p, tt, k, b] = x[b, CHSZ*ch+tt, 128k+p]
        arr = np.zeros((nch * CHSZ, D_IN, BL), np.float32)
        arr[:l_steps] = xl.transpose(1, 2, 0)
        arr = arr.reshape(nch, CHSZ, KI, 128, BL).transpose(0, 3, 1, 2, 4)
        m["xTT"] = np.ascontiguousarray(arr).reshape(
            nch * 128, CHSZ * KI * BL
        )
        in_maps.append(m)
    res = run_bass_kernel_spmd(
        nc,
        in_maps,
        core_ids=list(range(NCORES)),
        trace=trace,
        tmpdir=tmpdir,
    )
    out = np.concatenate([r["out"] for r in res.results], axis=0)
    return out, res


def kernel(**inputs) -> np.ndarray:
    out, _ = run(inputs)
    return out
